# revision 7
# baseline (speedup 1.0000x reference)
"""Trainium2 Bass kernel for nn_Attention_47725676593424.

GQA attention layer: B=2, S=2048, D=1024, H=16 q-heads, KVH=4 kv-heads,
HD=64, RoPE, causal mask, returns (out, new_k, new_v).

Sharding (8 cores): core = b*4 + g, b = batch (data parallel), g = head
group (tensor parallel). Each core computes q-heads [4g, 4g+4) and kv
head g for batch b (whole KV group local, GQA repeat is implicit), then
AllGathers the per-head attention output within its 4-core batch group
and computes a 256-column slice of the o_proj output.

Layout: activations flow transposed (feature on partitions, sequence on
the free axis) so every matmul contracts on the partition dim with zero
on-device transposes of x. Scores are computed transposed S^T[k, q]; the
softmax denominator comes free from a ones-column appended to V. The
causal mask is applied structurally (upper-triangular key blocks are
skipped / zeroed), which matches the reference's additive -1e9 mask
exactly because exp underflows to 0. Softmax runs without max
subtraction: scores*scale is bounded (|s| < ~4) for any plausible
activation scale here, so exp cannot overflow.
"""

import os
import numpy as np
from contextlib import ExitStack

import concourse.bass as bass
import concourse.mybir as mybir
import concourse.tile as tile
from concourse import bacc
from concourse.bass_utils import run_bass_kernel_spmd
from concourse.masks import make_identity

# Problem constants (hardcoded per harness contract).
B, S, D = 2, 2048, 1024
H, KVH, HD = 16, 4, 64
NCORES = 8
G = 4                 # head groups (tensor-parallel degree per batch)
HPG = H // G          # 4 q heads per core
DQ = HPG * HD         # 256 = per-core q/attn feature dim
SCALE = 1.0 / 8.0     # 1/sqrt(HD)
PAN = 512             # q panel width (one PSUM bank of fp32)
NPAN = S // PAN       # 4
SB = 128              # s block
NSB = S // SB         # 16
FCH = D // 128        # 8 feature chunks of the contraction dim
DT = mybir.dt.float32
F32 = np.float32

_NC_CACHE = {}


def _build_nc():
    nc = bacc.Bacc("TRN2", target_bir_lowering=False, debug=False,
                   num_devices=NCORES)

    xT_h = nc.dram_tensor("xT", [D, S], DT, kind="ExternalInput")
    cosT_h = nc.dram_tensor("cosT", [HD // 2, S], DT, kind="ExternalInput")
    sinT_h = nc.dram_tensor("sinT", [HD // 2, S], DT, kind="ExternalInput")
    wq_h = nc.dram_tensor("wq", [D, DQ], DT, kind="ExternalInput")
    wk_h = nc.dram_tensor("wk", [D, HD], DT, kind="ExternalInput")
    wv_h = nc.dram_tensor("wv", [D, HD], DT, kind="ExternalInput")
    wo_h = nc.dram_tensor("wo", [D, DQ], DT, kind="ExternalInput")
    out_h = nc.dram_tensor("out_s", [S, DQ], DT, kind="ExternalOutput")
    kout_h = nc.dram_tensor("k_out", [S, HD], DT, kind="ExternalOutput")
    vout_h = nc.dram_tensor("v_out", [S, HD], DT, kind="ExternalOutput")

    xT, cosT, sinT = xT_h.ap(), cosT_h.ap(), sinT_h.ap()
    wq, wk, wv, wo = wq_h.ap(), wk_h.ap(), wv_h.ap(), wo_h.ap()
    out_s, k_out, v_out = out_h.ap(), kout_h.ap(), vout_h.ap()

    with ExitStack() as ctx:
        tc = ctx.enter_context(tile.TileContext(nc))
        _emit(ctx, tc, nc, xT, cosT, sinT, wq, wk, wv, wo,
              out_s, k_out, v_out)

    nc.compile()
    return nc


def _emit(ctx, tc, nc, xT, cosT, sinT, wq, wk, wv, wo, out_s, k_out, v_out):
    EXP = mybir.ActivationFunctionType.Exp

    consts = ctx.enter_context(tc.tile_pool(name="consts", bufs=1))
    big = ctx.enter_context(tc.tile_pool(name="big", bufs=8))
    qkv = ctx.enter_context(tc.tile_pool(name="qkv", bufs=1))
    pt_pool = ctx.enter_context(tc.tile_pool(name="pt", bufs=3))
    tmp = ctx.enter_context(tc.tile_pool(name="tmp", bufs=6))
    outp = ctx.enter_context(tc.tile_pool(name="outp", bufs=3))
    dram = ctx.enter_context(tc.tile_pool(name="dram", bufs=1, space="DRAM"))

    # ---- constants / weights into SBUF ------------------------------------
    cosT_sb = consts.tile([HD // 2, S], DT)
    sinT_sb = consts.tile([HD // 2, S], DT)
    nc.sync.dma_start(out=cosT_sb, in_=cosT)
    nc.sync.dma_start(out=sinT_sb, in_=sinT)

    ident = consts.tile([128, 128], DT)
    make_identity(nc, ident)

    wq_sb = consts.tile([128, FCH, DQ], DT)
    wk_sb = consts.tile([128, FCH, HD], DT)
    wv_sb = consts.tile([128, FCH, HD], DT)
    wo_sb = consts.tile([128, FCH, DQ], DT)
    nc.sync.dma_start(out=wq_sb, in_=wq.rearrange("(c p) d -> p c d", p=128))
    nc.sync.dma_start(out=wk_sb, in_=wk.rearrange("(c p) d -> p c d", p=128))
    nc.sync.dma_start(out=wv_sb, in_=wv.rearrange("(c p) d -> p c d", p=128))
    nc.sync.dma_start(out=wo_sb, in_=wo.rearrange("(c p) d -> p c d", p=128))

    xt_sb = []
    for c in range(FCH):
        t = big.tile([128, S], DT, name=f"xt{c}", tag="big")
        nc.sync.dma_start(out=t, in_=xT[c * 128:(c + 1) * 128, :])
        xt_sb.append(t)

    # Persistent transposed activations.
    qT_sb = qkv.tile([128, 2, S], DT)       # 2 packs x (2 heads x 64)
    # kT duplicated in both partition halves so scores lhsT can match the
    # base partition (0 or 64) of each q head's rhs slice.
    kT_sb = qkv.tile([128, S], DT)
    vT_sb = qkv.tile([HD, S], DT)           # pre-transpose v
    v_ext = qkv.tile([128, NSB, 65], DT)    # v natural + ones column
    att0 = qkv.tile([128, S], DT)           # attn out^T, heads 0,1
    att1 = qkv.tile([128, S], DT)           # attn out^T, heads 2,3

    def rope_halves(dst, src, sp, base):
        """dst/src: [*, PAN] APs; rotate halves at partition base/base+32."""
        sl = slice(sp * PAN, (sp + 1) * PAN)
        c_sl = cosT_sb[:, sl]
        s_sl = sinT_sb[:, sl]
        a = slice(base, base + 32)
        b = slice(base + 32, base + 64)
        t1 = tmp.tile([32, PAN], DT, name="rt1", tag="ropet")
        t2 = tmp.tile([32, PAN], DT, name="rt2", tag="ropet")
        nc.vector.tensor_mul(t1, src[a], c_sl)
        nc.vector.tensor_mul(t2, src[b], s_sl)
        nc.vector.tensor_sub(dst[a], t1, t2)
        t3 = tmp.tile([32, PAN], DT, name="rt3", tag="ropet")
        t4 = tmp.tile([32, PAN], DT, name="rt4", tag="ropet")
        nc.vector.tensor_mul(t3, src[b], c_sl)
        nc.vector.tensor_mul(t4, src[a], s_sl)
        nc.vector.tensor_add(dst[b], t3, t4)

    # ---- phase 1: QKV projections + RoPE (own PSUM pool scope) ------------
    with tc.tile_pool(name="psA", bufs=2, space="PSUM") as psA:
        for pk in range(2):
            for sp in range(NPAN):
                q_ps = psA.tile([128, PAN], DT, name="q_ps", tag="ps")
                for c in range(FCH):
                    nc.tensor.matmul(
                        q_ps,
                        wq_sb[:, c, pk * 128:(pk + 1) * 128],
                        xt_sb[c][:, sp * PAN:(sp + 1) * PAN],
                        start=(c == 0), stop=(c == FCH - 1))
                dst = qT_sb[:, pk, sp * PAN:(sp + 1) * PAN]
                for hh in range(2):
                    rope_halves(dst, q_ps, sp, hh * 64)

        for sp in range(NPAN):
            k_ps = psA.tile([HD, PAN], DT, name="k_ps", tag="ps")
            for c in range(FCH):
                nc.tensor.matmul(
                    k_ps, wk_sb[:, c, :],
                    xt_sb[c][:, sp * PAN:(sp + 1) * PAN],
                    start=(c == 0), stop=(c == FCH - 1))
            rope_halves(kT_sb[0:HD, sp * PAN:(sp + 1) * PAN], k_ps, sp, 0)
        nc.sync.dma_start(out=kT_sb[HD:128, :], in_=kT_sb[0:HD, :])

        for sp in range(NPAN):
            v_ps = psA.tile([HD, PAN], DT, name="v_ps", tag="ps")
            for c in range(FCH):
                nc.tensor.matmul(
                    v_ps, wv_sb[:, c, :],
                    xt_sb[c][:, sp * PAN:(sp + 1) * PAN],
                    start=(c == 0), stop=(c == FCH - 1))
            nc.scalar.copy(vT_sb[:, sp * PAN:(sp + 1) * PAN], v_ps)

        # ---- phase 2: k/v back to natural layout for outputs + AV --------
        nc.vector.memset(v_ext[:, :, 64:65], 1.0)
        for kb in range(NSB):
            sl = slice(kb * 128, (kb + 1) * 128)
            vt_ps = psA.tile([128, HD], DT, name="vt_ps", tag="tp")
            nc.tensor.transpose(vt_ps, vT_sb[:, sl], ident[0:HD, 0:HD])
            nc.vector.tensor_copy(v_ext[:, kb, 0:HD], vt_ps)
            nc.sync.dma_start(out=v_out[sl, :], in_=v_ext[:, kb, 0:HD])

            kt_ps = psA.tile([128, HD], DT, name="kt_ps", tag="tp")
            nc.tensor.transpose(kt_ps, kT_sb[0:HD, sl], ident[0:HD, 0:HD])
            kn_sb = outp.tile([128, HD], DT, name="kn_sb", tag="kn")
            nc.vector.tensor_copy(kn_sb, kt_ps)
            nc.sync.dma_start(out=k_out[sl, :], in_=kn_sb)

    # ---- phase 3: causal attention, scores transposed --------------------
    with tc.tile_pool(name="psS", bufs=2, space="PSUM") as psS, \
         tc.tile_pool(name="psAV", bufs=1, space="PSUM") as psAV:
        for p in range(NPAN):
            nkb = 4 * (p + 1)
            q_sl = slice(p * PAN, (p + 1) * PAN)
            av_ps = psAV.tile([65, HPG, PAN], DT, name="av_ps", tag="av")
            for kb in range(nkb):
                k_sl = slice(kb * 128, (kb + 1) * 128)
                off = (kb - 4 * p) * 128  # >=0 only for diagonal blocks
                for hp in range(2):
                    s_ps = psS.tile([128, 2, PAN], DT, name="s_ps", tag="s")
                    for hi in range(2):
                        h = hp * 2 + hi
                        base = (h % 2) * 64
                        nc.tensor.matmul(
                            s_ps[:, hi, :],
                            kT_sb[base:base + 64, k_sl],
                            qT_sb[base:base + 64, h // 2, q_sl],
                            start=True, stop=True)
                    pt = pt_pool.tile([128, 2, PAN], DT, name="pt", tag="pt")
                    nc.scalar.activation(pt, s_ps, EXP, scale=SCALE)
                    if off >= 0:
                        for hi in range(2):
                            if off > 0:
                                nc.gpsimd.memset(pt[:, hi, 0:off], 0.0)
                            # keep q >= k within the diagonal 128-block
                            nc.gpsimd.affine_select(
                                out=pt[:, hi, off:off + 128],
                                in_=pt[:, hi, off:off + 128],
                                compare_op=mybir.AluOpType.is_ge,
                                fill=0.0, base=0,
                                pattern=[[1, 128]], channel_multiplier=-1)
                    for hi in range(2):
                        h = hp * 2 + hi
                        nc.tensor.matmul(
                            av_ps[:, h, :], v_ext[:, kb, :], pt[:, hi, :],
                            start=(kb == 0), stop=(kb == nkb - 1))
            for h in range(HPG):
                r_sb = tmp.tile([1, PAN], DT, name="r_sb", tag="r")
                nc.vector.reciprocal(r_sb, av_ps[64:65, h, :])
                rb = tmp.tile([HD, PAN], DT, name="rb", tag="rb")
                nc.gpsimd.partition_broadcast(rb, r_sb)
                att = att0 if h < 2 else att1
                nc.vector.tensor_mul(
                    att[(h % 2) * 64:(h % 2) * 64 + 64, q_sl],
                    av_ps[0:HD, h, :], rb)

    # ---- phase 4: AllGather attn^T across the 4-core batch group ---------
    cc_in = dram.tile([2, 128, S], DT)
    cc_out = dram.tile([G, 2, 128, S], DT)
    nc.sync.dma_start(out=cc_in[0], in_=att0)
    nc.sync.dma_start(out=cc_in[1], in_=att1)
    nc.gpsimd.collective_compute(
        "AllGather", mybir.AluOpType.bypass,
        replica_groups=[[0, 1, 2, 3], [4, 5, 6, 7]],
        ins=[cc_in.opt()], outs=[cc_out.opt()])

    at_sb = []
    for c in range(FCH):
        t = big.tile([128, S], DT, name=f"at{c}", tag="big")
        nc.sync.dma_start(out=t, in_=cc_out[c // 2, c % 2])
        at_sb.append(t)

    # ---- phase 5: o_proj column slice ------------------------------------
    with tc.tile_pool(name="psO", bufs=2, space="PSUM") as psO:
        for sb_i in range(NSB):
            sl = slice(sb_i * 128, (sb_i + 1) * 128)
            o_ps = psO.tile([128, DQ], DT, name="o_ps", tag="o")
            for c in range(FCH):
                nc.tensor.matmul(o_ps, at_sb[c][:, sl], wo_sb[:, c, :],
                                 start=(c == 0), stop=(c == FCH - 1))
            o_sb = outp.tile([128, DQ], DT, name="o_sb", tag="o_sb")
            nc.scalar.copy(o_sb, o_ps)
            nc.sync.dma_start(out=out_s[sl, :], in_=o_sb)


def get_nc():
    if "nc" not in _NC_CACHE:
        _NC_CACHE["nc"] = _build_nc()
    return _NC_CACHE["nc"]


def make_in_maps(x, cos, sin, wq, wk, wv, wo):
    cosT = np.ascontiguousarray(np.asarray(cos, F32).T)
    sinT = np.ascontiguousarray(np.asarray(sin, F32).T)
    x = np.asarray(x, F32)
    wq, wk, wv, wo = (np.asarray(a, F32) for a in (wq, wk, wv, wo))
    in_maps = []
    for core in range(NCORES):
        b, g = divmod(core, G)
        in_maps.append({
            "xT": np.ascontiguousarray(x[b].T),
            "cosT": cosT,
            "sinT": sinT,
            "wq": np.ascontiguousarray(wq[:, g * DQ:(g + 1) * DQ]),
            "wk": np.ascontiguousarray(wk[:, g * HD:(g + 1) * HD]),
            "wv": np.ascontiguousarray(wv[:, g * HD:(g + 1) * HD]),
            "wo": np.ascontiguousarray(wo[:, g * DQ:(g + 1) * DQ]),
        })
    return in_maps


def assemble(results):
    out = np.empty((B, S, D), F32)
    new_k = np.empty((B, S, KVH, HD), F32)
    new_v = np.empty((B, S, KVH, HD), F32)
    for core in range(NCORES):
        b, g = divmod(core, G)
        r = results[core]
        out[b, :, g * DQ:(g + 1) * DQ] = r["out_s"]
        new_k[b, :, g, :] = r["k_out"]
        new_v[b, :, g, :] = r["v_out"]
    return out, new_k, new_v


def _ensure_ntff_hook():
    """Register the axon NTFF profile hook if the container's antenv stub
    lacks it (needed only for trace=True timing runs)."""
    import sys
    import types
    try:
        from antenv.axon_hooks import get_axon_ntff_profile_hook  # noqa: F401
        return
    except ImportError:
        pass
    try:
        import antenv
        from trn_agent_boot.trn_boot import _ntff_profile_via_ctypes
        mod = types.ModuleType("antenv.axon_hooks")
        state = {"fn": None}
        mod.set_axon_ntff_profile_hook = lambda fn: state.update(fn=fn)
        mod.get_axon_ntff_profile_hook = lambda: state["fn"]
        sys.modules["antenv.axon_hooks"] = mod
        antenv.axon_hooks = mod
        hook = _ntff_profile_via_ctypes("/opt/axon/libaxon_pjrt.so")
        if hook is not None:
            mod.set_axon_ntff_profile_hook(hook)
    except Exception as e:  # profiling is best-effort; never break the run
        print(f"ntff hook setup failed: {e}")


def kernel(x, cos, sin, mask, wq, wk, wv, wo):
    # mask is not shipped to the device: the kernel applies causality
    # structurally, which matches the reference's -1e9 additive mask.
    nc = get_nc()
    in_maps = make_in_maps(x, cos, sin, wq, wk, wv, wo)
    trace = bool(int(os.environ.get("KERNEL_TRACE", "0")))
    if trace:
        _ensure_ntff_hook()
    res = run_bass_kernel_spmd(nc, in_maps, list(range(NCORES)), trace=trace)
    if trace:
        _NC_CACHE["last_exec_time_ns"] = res.exec_time_ns
    return assemble(res.results)


# revision 8
# speedup vs baseline: 1.8799x; 1.8799x over previous
"""Trainium2 Bass kernel for nn_Attention_47725676593424.

GQA attention layer: B=2, S=2048, D=1024, H=16 q-heads, KVH=4 kv-heads,
HD=64, RoPE, causal mask, returns (out, new_k, new_v).

Sharding (8 cores): core = b*4 + g, b = batch (data parallel), g = head
group (tensor parallel). Each core computes q-heads [4g, 4g+4) and kv
head g for batch b (whole KV group local, GQA repeat is implicit), then
AllGathers the per-head attention output within its 4-core batch group
and computes a 256-column slice of the o_proj output.

Layout: activations flow transposed (feature on partitions, sequence on
the free axis) so every matmul contracts on the partition dim with zero
on-device transposes of x. Scores are computed transposed S^T[k, q]; the
softmax denominator comes free from a ones-column appended to V. The
causal mask is applied structurally (upper-triangular key blocks are
skipped / zeroed), which matches the reference's additive -1e9 mask
exactly because exp underflows to 0. Softmax runs without max
subtraction: scores*scale is bounded (|s| < ~4) for any plausible
activation scale here, so exp cannot overflow.
"""

import os
import numpy as np
from contextlib import ExitStack

import concourse.bass as bass
import concourse.mybir as mybir
import concourse.tile as tile
from concourse import bacc
from concourse.bass_utils import run_bass_kernel_spmd
from concourse.masks import make_identity

# Problem constants (hardcoded per harness contract).
B, S, D = 2, 2048, 1024
H, KVH, HD = 16, 4, 64
NCORES = 8
G = 4                 # head groups (tensor-parallel degree per batch)
HPG = H // G          # 4 q heads per core
DQ = HPG * HD         # 256 = per-core q/attn feature dim
SCALE = 1.0 / 8.0     # 1/sqrt(HD)
PAN = 512             # q panel width (one PSUM bank of fp32)
NPAN = S // PAN       # 4
SB = 128              # s block
NSB = S // SB         # 16
FCH = D // 128        # 8 feature chunks of the contraction dim
DT = mybir.dt.float32
BF = mybir.dt.bfloat16
F32 = np.float32
try:
    import ml_dtypes
    NPBF = ml_dtypes.bfloat16
except ImportError:  # pragma: no cover
    NPBF = None

_NC_CACHE = {}


def _build_nc():
    nc = bacc.Bacc("TRN2", target_bir_lowering=False, debug=False,
                   num_devices=NCORES)

    xT_h = nc.dram_tensor("xT", [D, S], BF, kind="ExternalInput")
    cosT_h = nc.dram_tensor("cosT", [HD // 2, S], DT, kind="ExternalInput")
    sinT_h = nc.dram_tensor("sinT", [HD // 2, S], DT, kind="ExternalInput")
    wq_h = nc.dram_tensor("wq", [D, DQ], BF, kind="ExternalInput")
    wk_h = nc.dram_tensor("wk", [D, HD], BF, kind="ExternalInput")
    wv_h = nc.dram_tensor("wv", [D, HD], BF, kind="ExternalInput")
    wo_h = nc.dram_tensor("wo", [D, DQ], BF, kind="ExternalInput")
    out_h = nc.dram_tensor("out_s", [S, DQ], DT, kind="ExternalOutput")
    kout_h = nc.dram_tensor("k_out", [S, HD], DT, kind="ExternalOutput")
    vout_h = nc.dram_tensor("v_out", [S, HD], DT, kind="ExternalOutput")

    xT, cosT, sinT = xT_h.ap(), cosT_h.ap(), sinT_h.ap()
    wq, wk, wv, wo = wq_h.ap(), wk_h.ap(), wv_h.ap(), wo_h.ap()
    out_s, k_out, v_out = out_h.ap(), kout_h.ap(), vout_h.ap()

    with ExitStack() as ctx:
        tc = ctx.enter_context(tile.TileContext(nc))
        _emit(ctx, tc, nc, xT, cosT, sinT, wq, wk, wv, wo,
              out_s, k_out, v_out)

    nc.compile()
    return nc


def _emit(ctx, tc, nc, xT, cosT, sinT, wq, wk, wv, wo, out_s, k_out, v_out):
    EXP = mybir.ActivationFunctionType.Exp

    consts = ctx.enter_context(tc.tile_pool(name="consts", bufs=1))
    big = ctx.enter_context(tc.tile_pool(name="big", bufs=8))
    qkv = ctx.enter_context(tc.tile_pool(name="qkv", bufs=1))
    pt_pool = ctx.enter_context(tc.tile_pool(name="pt", bufs=3))
    tmp = ctx.enter_context(tc.tile_pool(name="tmp", bufs=6))
    outp = ctx.enter_context(tc.tile_pool(name="outp", bufs=3))
    dram = ctx.enter_context(tc.tile_pool(name="dram", bufs=1, space="DRAM"))

    # ---- constants / weights into SBUF ------------------------------------
    cosT_sb = consts.tile([HD // 2, S], DT)
    sinT_sb = consts.tile([HD // 2, S], DT)
    nc.sync.dma_start(out=cosT_sb, in_=cosT)
    nc.sync.dma_start(out=sinT_sb, in_=sinT)

    ident = consts.tile([128, 128], DT)
    make_identity(nc, ident)

    wq_sb = consts.tile([128, FCH, DQ], BF)
    wk_sb = consts.tile([128, FCH, HD], BF)
    wv_sb = consts.tile([128, FCH, HD], BF)
    wo_sb = consts.tile([128, FCH, DQ], BF)
    nc.sync.dma_start(out=wq_sb, in_=wq.rearrange("(c p) d -> p c d", p=128))
    nc.sync.dma_start(out=wk_sb, in_=wk.rearrange("(c p) d -> p c d", p=128))
    nc.sync.dma_start(out=wv_sb, in_=wv.rearrange("(c p) d -> p c d", p=128))
    nc.sync.dma_start(out=wo_sb, in_=wo.rearrange("(c p) d -> p c d", p=128))

    xt_sb = []
    for c in range(FCH):
        t = big.tile([128, S], BF, name=f"xt{c}", tag="big")
        nc.sync.dma_start(out=t, in_=xT[c * 128:(c + 1) * 128, :])
        xt_sb.append(t)

    # Persistent transposed activations.
    qT_sb = qkv.tile([128, 2, S], BF)       # 2 packs x (2 heads x 64)
    # k master in fp32 (feeds the k_out output); bf16 copy duplicated in
    # both partition halves so scores lhsT can match the base partition
    # (0 or 64) of each q head's rhs slice.
    kT_f32 = qkv.tile([HD, S], DT)
    kT_bf = qkv.tile([128, S], BF)
    vT_sb = qkv.tile([HD, S], DT)           # pre-transpose v (fp32 master)
    v_ext = qkv.tile([128, NSB, 65], DT)    # v natural + ones column
    v_ext_bf = qkv.tile([128, NSB, 65], BF)
    att0 = qkv.tile([128, S], BF)           # attn out^T, heads 0,1
    att1 = qkv.tile([128, S], BF)           # attn out^T, heads 2,3

    def rope_halves(dst, src, sp, base):
        """dst/src: [*, PAN] APs; rotate halves at partition base/base+32."""
        sl = slice(sp * PAN, (sp + 1) * PAN)
        c_sl = cosT_sb[:, sl]
        s_sl = sinT_sb[:, sl]
        a = slice(base, base + 32)
        b = slice(base + 32, base + 64)
        t1 = tmp.tile([32, PAN], DT, name="rt1", tag="ropet")
        t2 = tmp.tile([32, PAN], DT, name="rt2", tag="ropet")
        nc.vector.tensor_mul(t1, src[a], c_sl)
        nc.vector.tensor_mul(t2, src[b], s_sl)
        nc.vector.tensor_sub(dst[a], t1, t2)
        t3 = tmp.tile([32, PAN], DT, name="rt3", tag="ropet")
        t4 = tmp.tile([32, PAN], DT, name="rt4", tag="ropet")
        nc.vector.tensor_mul(t3, src[b], c_sl)
        nc.vector.tensor_mul(t4, src[a], s_sl)
        nc.vector.tensor_add(dst[b], t3, t4)

    # ---- phase 1: QKV projections + RoPE (own PSUM pool scope) ------------
    with tc.tile_pool(name="psA", bufs=2, space="PSUM") as psA:
        for pk in range(2):
            for sp in range(NPAN):
                q_ps = psA.tile([128, PAN], DT, name="q_ps", tag="ps")
                for c in range(FCH):
                    nc.tensor.matmul(
                        q_ps,
                        wq_sb[:, c, pk * 128:(pk + 1) * 128],
                        xt_sb[c][:, sp * PAN:(sp + 1) * PAN],
                        start=(c == 0), stop=(c == FCH - 1))
                dst = qT_sb[:, pk, sp * PAN:(sp + 1) * PAN]
                for hh in range(2):
                    rope_halves(dst, q_ps, sp, hh * 64)

        for sp in range(NPAN):
            k_ps = psA.tile([HD, PAN], DT, name="k_ps", tag="ps")
            for c in range(FCH):
                nc.tensor.matmul(
                    k_ps, wk_sb[:, c, :],
                    xt_sb[c][:, sp * PAN:(sp + 1) * PAN],
                    start=(c == 0), stop=(c == FCH - 1))
            rope_halves(kT_f32[:, sp * PAN:(sp + 1) * PAN], k_ps, sp, 0)
        nc.vector.tensor_copy(kT_bf[0:HD, :], kT_f32)
        nc.sync.dma_start(out=kT_bf[HD:128, :], in_=kT_bf[0:HD, :])

        for sp in range(NPAN):
            v_ps = psA.tile([HD, PAN], DT, name="v_ps", tag="ps")
            for c in range(FCH):
                nc.tensor.matmul(
                    v_ps, wv_sb[:, c, :],
                    xt_sb[c][:, sp * PAN:(sp + 1) * PAN],
                    start=(c == 0), stop=(c == FCH - 1))
            nc.scalar.copy(vT_sb[:, sp * PAN:(sp + 1) * PAN], v_ps)

        # ---- phase 2: k/v back to natural layout for outputs + AV --------
        nc.vector.memset(v_ext[:, :, 64:65], 1.0)
        for kb in range(NSB):
            sl = slice(kb * 128, (kb + 1) * 128)
            vt_ps = psA.tile([128, HD], DT, name="vt_ps", tag="tp")
            nc.tensor.transpose(vt_ps, vT_sb[:, sl], ident[0:HD, 0:HD])
            nc.vector.tensor_copy(v_ext[:, kb, 0:HD], vt_ps)
            nc.sync.dma_start(out=v_out[sl, :], in_=v_ext[:, kb, 0:HD])

            kt_ps = psA.tile([128, HD], DT, name="kt_ps", tag="tp")
            nc.tensor.transpose(kt_ps, kT_f32[:, sl], ident[0:HD, 0:HD])
            kn_sb = outp.tile([128, HD], DT, name="kn_sb", tag="kn")
            nc.vector.tensor_copy(kn_sb, kt_ps)
            nc.sync.dma_start(out=k_out[sl, :], in_=kn_sb)
        nc.vector.tensor_copy(v_ext_bf, v_ext)

    # ---- phase 3: causal attention, scores transposed --------------------
    with tc.tile_pool(name="psS", bufs=2, space="PSUM") as psS, \
         tc.tile_pool(name="psAV", bufs=1, space="PSUM") as psAV:
        for p in range(NPAN):
            nkb = 4 * (p + 1)
            q_sl = slice(p * PAN, (p + 1) * PAN)
            av_ps = psAV.tile([65, HPG, PAN], DT, name="av_ps", tag="av")
            for kb in range(nkb):
                k_sl = slice(kb * 128, (kb + 1) * 128)
                off = (kb - 4 * p) * 128  # >=0 only for diagonal blocks
                for hp in range(2):
                    s_ps = psS.tile([128, 2, PAN], DT, name="s_ps", tag="s")
                    for hi in range(2):
                        h = hp * 2 + hi
                        base = (h % 2) * 64
                        nc.tensor.matmul(
                            s_ps[:, hi, :],
                            kT_bf[base:base + 64, k_sl],
                            qT_sb[base:base + 64, h // 2, q_sl],
                            start=True, stop=True)
                    pt = pt_pool.tile([128, 2, PAN], BF, name="pt", tag="pt")
                    nc.scalar.activation(pt, s_ps, EXP, scale=SCALE)
                    if off >= 0:
                        for hi in range(2):
                            if off > 0:
                                nc.gpsimd.memset(pt[:, hi, 0:off], 0.0)
                            # keep q >= k within the diagonal 128-block
                            nc.gpsimd.affine_select(
                                out=pt[:, hi, off:off + 128],
                                in_=pt[:, hi, off:off + 128],
                                compare_op=mybir.AluOpType.is_ge,
                                fill=0.0, base=0,
                                pattern=[[1, 128]], channel_multiplier=-1)
                    for hi in range(2):
                        h = hp * 2 + hi
                        nc.tensor.matmul(
                            av_ps[:, h, :], v_ext_bf[:, kb, :], pt[:, hi, :],
                            start=(kb == 0), stop=(kb == nkb - 1))
            for h in range(HPG):
                r_sb = tmp.tile([1, PAN], DT, name="r_sb", tag="r")
                nc.vector.reciprocal(r_sb, av_ps[64:65, h, :])
                rb = tmp.tile([HD, PAN], DT, name="rb", tag="rb")
                nc.gpsimd.partition_broadcast(rb, r_sb)
                att = att0 if h < 2 else att1
                nc.vector.tensor_mul(
                    att[(h % 2) * 64:(h % 2) * 64 + 64, q_sl],
                    av_ps[0:HD, h, :], rb)

    # ---- phase 4: AllGather attn^T across the 4-core batch group ---------
    cc_in = dram.tile([2, 128, S], BF)
    cc_out = dram.tile([G, 2, 128, S], BF)
    nc.sync.dma_start(out=cc_in[0], in_=att0)
    nc.sync.dma_start(out=cc_in[1], in_=att1)
    nc.gpsimd.collective_compute(
        "AllGather", mybir.AluOpType.bypass,
        replica_groups=[[0, 1, 2, 3], [4, 5, 6, 7]],
        ins=[cc_in.opt()], outs=[cc_out.opt()])

    at_sb = []
    for c in range(FCH):
        t = big.tile([128, S], BF, name=f"at{c}", tag="big")
        nc.sync.dma_start(out=t, in_=cc_out[c // 2, c % 2])
        at_sb.append(t)

    # ---- phase 5: o_proj column slice ------------------------------------
    with tc.tile_pool(name="psO", bufs=2, space="PSUM") as psO:
        for sb_i in range(NSB):
            sl = slice(sb_i * 128, (sb_i + 1) * 128)
            o_ps = psO.tile([128, DQ], DT, name="o_ps", tag="o")
            for c in range(FCH):
                nc.tensor.matmul(o_ps, at_sb[c][:, sl], wo_sb[:, c, :],
                                 start=(c == 0), stop=(c == FCH - 1))
            o_sb = outp.tile([128, DQ], DT, name="o_sb", tag="o_sb")
            nc.scalar.copy(o_sb, o_ps)
            nc.sync.dma_start(out=out_s[sl, :], in_=o_sb)


def get_nc():
    if "nc" not in _NC_CACHE:
        _NC_CACHE["nc"] = _build_nc()
    return _NC_CACHE["nc"]


def make_in_maps(x, cos, sin, wq, wk, wv, wo):
    cosT = np.ascontiguousarray(np.asarray(cos, F32).T)
    sinT = np.ascontiguousarray(np.asarray(sin, F32).T)
    x = np.asarray(x, F32).astype(NPBF)
    wq, wk, wv, wo = (np.asarray(a, F32).astype(NPBF)
                      for a in (wq, wk, wv, wo))
    in_maps = []
    for core in range(NCORES):
        b, g = divmod(core, G)
        in_maps.append({
            "xT": np.ascontiguousarray(x[b].T),
            "cosT": cosT,
            "sinT": sinT,
            "wq": np.ascontiguousarray(wq[:, g * DQ:(g + 1) * DQ]),
            "wk": np.ascontiguousarray(wk[:, g * HD:(g + 1) * HD]),
            "wv": np.ascontiguousarray(wv[:, g * HD:(g + 1) * HD]),
            "wo": np.ascontiguousarray(wo[:, g * DQ:(g + 1) * DQ]),
        })
    return in_maps


def assemble(results):
    out = np.empty((B, S, D), F32)
    new_k = np.empty((B, S, KVH, HD), F32)
    new_v = np.empty((B, S, KVH, HD), F32)
    for core in range(NCORES):
        b, g = divmod(core, G)
        r = results[core]
        out[b, :, g * DQ:(g + 1) * DQ] = r["out_s"]
        new_k[b, :, g, :] = r["k_out"]
        new_v[b, :, g, :] = r["v_out"]
    return out, new_k, new_v


def _ensure_ntff_hook():
    """Register the axon NTFF profile hook if the container's antenv stub
    lacks it (needed only for trace=True timing runs)."""
    import sys
    import types
    try:
        from antenv.axon_hooks import get_axon_ntff_profile_hook  # noqa: F401
        return
    except ImportError:
        pass
    try:
        import antenv
        from trn_agent_boot.trn_boot import _ntff_profile_via_ctypes
        mod = types.ModuleType("antenv.axon_hooks")
        state = {"fn": None}
        mod.set_axon_ntff_profile_hook = lambda fn: state.update(fn=fn)
        mod.get_axon_ntff_profile_hook = lambda: state["fn"]
        sys.modules["antenv.axon_hooks"] = mod
        antenv.axon_hooks = mod
        hook = _ntff_profile_via_ctypes("/opt/axon/libaxon_pjrt.so")
        if hook is not None:
            mod.set_axon_ntff_profile_hook(hook)
    except Exception as e:  # profiling is best-effort; never break the run
        print(f"ntff hook setup failed: {e}")


def kernel(x, cos, sin, mask, wq, wk, wv, wo):
    # mask is not shipped to the device: the kernel applies causality
    # structurally, which matches the reference's -1e9 additive mask.
    nc = get_nc()
    in_maps = make_in_maps(x, cos, sin, wq, wk, wv, wo)
    trace = bool(int(os.environ.get("KERNEL_TRACE", "0")))
    if trace:
        _ensure_ntff_hook()
    res = run_bass_kernel_spmd(nc, in_maps, list(range(NCORES)), trace=trace)
    if trace:
        _NC_CACHE["last_exec_time_ns"] = res.exec_time_ns
    return assemble(res.results)


# revision 10
# speedup vs baseline: 2.3383x; 1.2438x over previous
"""Trainium2 Bass kernel for nn_Attention_47725676593424.

GQA attention layer: B=2, S=2048, D=1024, H=16 q-heads, KVH=4 kv-heads,
HD=64, RoPE, causal mask, returns (out, new_k, new_v).

Sharding (8 cores): core = b*4 + g, b = batch (data parallel), g = head
group (tensor parallel). Each core computes q-heads [4g, 4g+4) and kv
head g for batch b (whole KV group local, GQA repeat is implicit), then
AllGathers the per-head attention output within its 4-core batch group
and computes a 256-column slice of the o_proj output.

Layout: activations flow transposed (feature on partitions, sequence on
the free axis) so every matmul contracts on the partition dim with zero
on-device transposes of x. Scores are computed transposed S^T[k, q]; the
softmax denominator comes free from a ones-column appended to V. The
causal mask is applied structurally (upper-triangular key blocks are
skipped / zeroed), which matches the reference's additive -1e9 mask
exactly because exp underflows to 0. Softmax runs without max
subtraction: scores*scale is bounded (|s| < ~4) for any plausible
activation scale here, so exp cannot overflow.
"""

import os
import numpy as np
from contextlib import ExitStack

import concourse.bass as bass
import concourse.mybir as mybir
import concourse.tile as tile
from concourse import bacc
from concourse.bass_utils import run_bass_kernel_spmd
from concourse.masks import make_identity

# Problem constants (hardcoded per harness contract).
B, S, D = 2, 2048, 1024
H, KVH, HD = 16, 4, 64
NCORES = 8
G = 4                 # head groups (tensor-parallel degree per batch)
HPG = H // G          # 4 q heads per core
DQ = HPG * HD         # 256 = per-core q/attn feature dim
SCALE = 1.0 / 8.0     # 1/sqrt(HD)
PAN = 512             # q panel width (one PSUM bank of fp32)
NPAN = S // PAN       # 4
SB = 128              # s block
NSB = S // SB         # 16
FCH = D // 128        # 8 feature chunks of the contraction dim
DT = mybir.dt.float32
BF = mybir.dt.bfloat16
F32 = np.float32
try:
    import ml_dtypes
    NPBF = ml_dtypes.bfloat16
except ImportError:  # pragma: no cover
    NPBF = None

_NC_CACHE = {}


def _build_nc():
    nc = bacc.Bacc("TRN2", target_bir_lowering=False, debug=False,
                   num_devices=NCORES)

    xT_h = nc.dram_tensor("xT", [D, S], BF, kind="ExternalInput")
    cosT_h = nc.dram_tensor("cosT", [HD // 2, S], DT, kind="ExternalInput")
    sinT_h = nc.dram_tensor("sinT", [HD // 2, S], DT, kind="ExternalInput")
    wq_h = nc.dram_tensor("wq", [D, DQ], BF, kind="ExternalInput")
    wk_h = nc.dram_tensor("wk", [D, HD], BF, kind="ExternalInput")
    wv_h = nc.dram_tensor("wv", [D, HD], BF, kind="ExternalInput")
    wo_h = nc.dram_tensor("wo", [D, DQ], BF, kind="ExternalInput")
    out_h = nc.dram_tensor("out_s", [S, DQ], DT, kind="ExternalOutput")
    kout_h = nc.dram_tensor("k_out", [S, HD], DT, kind="ExternalOutput")
    vout_h = nc.dram_tensor("v_out", [S, HD], DT, kind="ExternalOutput")

    xT, cosT, sinT = xT_h.ap(), cosT_h.ap(), sinT_h.ap()
    wq, wk, wv, wo = wq_h.ap(), wk_h.ap(), wv_h.ap(), wo_h.ap()
    out_s, k_out, v_out = out_h.ap(), kout_h.ap(), vout_h.ap()

    with ExitStack() as ctx:
        tc = ctx.enter_context(tile.TileContext(nc))
        _emit(ctx, tc, nc, xT, cosT, sinT, wq, wk, wv, wo,
              out_s, k_out, v_out)

    nc.compile()
    return nc


def _emit(ctx, tc, nc, xT, cosT, sinT, wq, wk, wv, wo, out_s, k_out, v_out):
    EXP = mybir.ActivationFunctionType.Exp

    consts = ctx.enter_context(tc.tile_pool(name="consts", bufs=1))
    big = ctx.enter_context(tc.tile_pool(name="big", bufs=8))
    qkv = ctx.enter_context(tc.tile_pool(name="qkv", bufs=1))
    pt_pool = ctx.enter_context(tc.tile_pool(name="pt", bufs=3))
    tmp = ctx.enter_context(tc.tile_pool(name="tmp", bufs=6))
    outp = ctx.enter_context(tc.tile_pool(name="outp", bufs=3))
    dram = ctx.enter_context(tc.tile_pool(name="dram", bufs=1, space="DRAM"))

    # ---- constants / weights into SBUF ------------------------------------
    cosT_sb = consts.tile([HD // 2, S], DT)
    sinT_sb = consts.tile([HD // 2, S], DT)
    nc.sync.dma_start(out=cosT_sb, in_=cosT)
    nc.sync.dma_start(out=sinT_sb, in_=sinT)

    ident = consts.tile([128, 128], DT)
    make_identity(nc, ident)

    wq_sb = consts.tile([128, FCH, DQ], BF)
    wk_sb = consts.tile([128, FCH, HD], BF)
    wv_sb = consts.tile([128, FCH, HD], BF)
    wo_sb = consts.tile([128, FCH, DQ], BF)
    nc.sync.dma_start(out=wq_sb, in_=wq.rearrange("(c p) d -> p c d", p=128))
    nc.sync.dma_start(out=wk_sb, in_=wk.rearrange("(c p) d -> p c d", p=128))
    nc.sync.dma_start(out=wv_sb, in_=wv.rearrange("(c p) d -> p c d", p=128))
    nc.sync.dma_start(out=wo_sb, in_=wo.rearrange("(c p) d -> p c d", p=128))

    xt_sb = []
    for c in range(FCH):
        t = big.tile([128, S], BF, name=f"xt{c}", tag="big")
        nc.sync.dma_start(out=t, in_=xT[c * 128:(c + 1) * 128, :])
        xt_sb.append(t)

    # Persistent transposed activations.
    qT_sb = qkv.tile([128, 2, S], BF)       # 2 packs x (2 heads x 64)
    # k master in fp32 (feeds the k_out output); bf16 copy duplicated in
    # both partition halves so scores lhsT can match the base partition
    # (0 or 64) of each q head's rhs slice.
    kT_f32 = qkv.tile([HD, S], DT)
    kT_bf = qkv.tile([128, S], BF)
    vT_sb = qkv.tile([HD, S], DT)           # pre-transpose v (fp32 master)
    v_ext = qkv.tile([128, NSB, 65], DT)    # v natural + ones column
    v_ext_bf = qkv.tile([128, NSB, 65], BF)
    att0 = qkv.tile([128, S], BF)           # attn out^T, heads 0,1
    att1 = qkv.tile([128, S], BF)           # attn out^T, heads 2,3

    def rope_halves(dst, src, sp, base):
        """dst/src: [*, PAN] APs; rotate halves at partition base/base+32."""
        sl = slice(sp * PAN, (sp + 1) * PAN)
        c_sl = cosT_sb[:, sl]
        s_sl = sinT_sb[:, sl]
        a = slice(base, base + 32)
        b = slice(base + 32, base + 64)
        t1 = tmp.tile([32, PAN], DT, name="rt1", tag="ropet")
        t2 = tmp.tile([32, PAN], DT, name="rt2", tag="ropet")
        nc.vector.tensor_mul(t1, src[a], c_sl)
        nc.vector.tensor_mul(t2, src[b], s_sl)
        nc.vector.tensor_sub(dst[a], t1, t2)
        t3 = tmp.tile([32, PAN], DT, name="rt3", tag="ropet")
        t4 = tmp.tile([32, PAN], DT, name="rt4", tag="ropet")
        nc.vector.tensor_mul(t3, src[b], c_sl)
        nc.vector.tensor_mul(t4, src[a], s_sl)
        nc.vector.tensor_add(dst[b], t3, t4)

    # ---- phase 1: QKV projections + RoPE (own PSUM pool scope) ------------
    with tc.tile_pool(name="psA", bufs=2, space="PSUM") as psA:
        for pk in range(2):
            for sp in range(NPAN):
                q_ps = psA.tile([128, PAN], DT, name="q_ps", tag="ps")
                for c in range(FCH):
                    nc.tensor.matmul(
                        q_ps,
                        wq_sb[:, c, pk * 128:(pk + 1) * 128],
                        xt_sb[c][:, sp * PAN:(sp + 1) * PAN],
                        start=(c == 0), stop=(c == FCH - 1))
                dst = qT_sb[:, pk, sp * PAN:(sp + 1) * PAN]
                for hh in range(2):
                    rope_halves(dst, q_ps, sp, hh * 64)

        for sp in range(NPAN):
            k_ps = psA.tile([HD, PAN], DT, name="k_ps", tag="ps")
            for c in range(FCH):
                nc.tensor.matmul(
                    k_ps, wk_sb[:, c, :],
                    xt_sb[c][:, sp * PAN:(sp + 1) * PAN],
                    start=(c == 0), stop=(c == FCH - 1))
            rope_halves(kT_f32[:, sp * PAN:(sp + 1) * PAN], k_ps, sp, 0)
        nc.vector.tensor_copy(kT_bf[0:HD, :], kT_f32)
        nc.sync.dma_start(out=kT_bf[HD:128, :], in_=kT_bf[0:HD, :])

        for sp in range(NPAN):
            v_ps = psA.tile([HD, PAN], DT, name="v_ps", tag="ps")
            for c in range(FCH):
                nc.tensor.matmul(
                    v_ps, wv_sb[:, c, :],
                    xt_sb[c][:, sp * PAN:(sp + 1) * PAN],
                    start=(c == 0), stop=(c == FCH - 1))
            nc.scalar.copy(vT_sb[:, sp * PAN:(sp + 1) * PAN], v_ps)

        # ---- phase 2: k/v back to natural layout for outputs + AV --------
        nc.vector.memset(v_ext[:, :, 64:65], 1.0)
        for kb in range(NSB):
            sl = slice(kb * 128, (kb + 1) * 128)
            vt_ps = psA.tile([128, HD], DT, name="vt_ps", tag="tp")
            nc.tensor.transpose(vt_ps, vT_sb[:, sl], ident[0:HD, 0:HD])
            nc.vector.tensor_copy(v_ext[:, kb, 0:HD], vt_ps)
            nc.sync.dma_start(out=v_out[sl, :], in_=v_ext[:, kb, 0:HD])

            kt_ps = psA.tile([128, HD], DT, name="kt_ps", tag="tp")
            nc.tensor.transpose(kt_ps, kT_f32[:, sl], ident[0:HD, 0:HD])
            kn_sb = outp.tile([128, HD], DT, name="kn_sb", tag="kn")
            nc.vector.tensor_copy(kn_sb, kt_ps)
            nc.sync.dma_start(out=k_out[sl, :], in_=kn_sb)
        nc.vector.tensor_copy(v_ext_bf, v_ext)

    # ---- phase 3: causal attention, scores transposed --------------------
    # Head-pair outer so each half of attn^T finishes early and its
    # AllGather overlaps the other half's compute.
    cc_ins = [dram.tile([128, S], BF, name=f"cc_in{i}") for i in range(2)]
    cc_outs = [dram.tile([G, 128, S], BF, name=f"cc_out{i}") for i in range(2)]
    with tc.tile_pool(name="psS", bufs=2, space="PSUM") as psS, \
         tc.tile_pool(name="psAV", bufs=2, space="PSUM") as psAV:
        for hp in range(2):
            att = att0 if hp == 0 else att1
            for p in range(NPAN):
                nkb = 4 * (p + 1)
                q_sl = slice(p * PAN, (p + 1) * PAN)
                av_ps = psAV.tile([65, 2, PAN], DT, name="av_ps", tag="av")
                for kb in range(nkb):
                    k_sl = slice(kb * 128, (kb + 1) * 128)
                    off = (kb - 4 * p) * 128  # >=0 only on diagonal blocks
                    s_ps = psS.tile([128, 2, PAN], DT, name="s_ps", tag="s")
                    for hi in range(2):
                        base = hi * 64
                        nc.tensor.matmul(
                            s_ps[:, hi, :],
                            kT_bf[base:base + 64, k_sl],
                            qT_sb[base:base + 64, hp, q_sl],
                            start=True, stop=True)
                    pt = pt_pool.tile([128, 2, PAN], BF, name="pt", tag="pt")
                    if off > 0:
                        nc.scalar.activation(pt[:, :, off:], s_ps[:, :, off:],
                                             EXP, scale=SCALE)
                        nc.gpsimd.memset(pt[:, :, 0:off], 0.0)
                    else:
                        nc.scalar.activation(pt, s_ps, EXP, scale=SCALE)
                    if off >= 0:
                        for hi in range(2):
                            # keep q >= k within the diagonal 128-block
                            nc.gpsimd.affine_select(
                                out=pt[:, hi, off:off + 128],
                                in_=pt[:, hi, off:off + 128],
                                compare_op=mybir.AluOpType.is_ge,
                                fill=0.0, base=0,
                                pattern=[[1, 128]], channel_multiplier=-1)
                    for hi in range(2):
                        nc.tensor.matmul(
                            av_ps[:, hi, :], v_ext_bf[:, kb, :], pt[:, hi, :],
                            start=(kb == 0), stop=(kb == nkb - 1))
                for hi in range(2):
                    r_sb = tmp.tile([1, PAN], DT, name="r_sb", tag="r")
                    nc.vector.reciprocal(r_sb, av_ps[64:65, hi, :])
                    rb = tmp.tile([HD, PAN], DT, name="rb", tag="rb")
                    nc.gpsimd.partition_broadcast(rb, r_sb)
                    nc.vector.tensor_mul(
                        att[hi * 64:hi * 64 + 64, q_sl],
                        av_ps[0:HD, hi, :], rb)
            # ship this head-pair's attn^T while the next pair computes
            nc.sync.dma_start(out=cc_ins[hp], in_=att)
            nc.gpsimd.collective_compute(
                "AllGather", mybir.AluOpType.bypass,
                replica_groups=[[0, 1, 2, 3], [4, 5, 6, 7]],
                ins=[cc_ins[hp].opt()], outs=[cc_outs[hp].opt()])

    # cc_out[hp][r] holds heads {4r+2hp, 4r+2hp+1} = wo chunk 2r+hp.
    at_sb = [None] * FCH
    for hp in range(2):
        for r in range(G):
            t = big.tile([128, S], BF, name=f"at{hp}_{r}", tag="big")
            nc.sync.dma_start(out=t, in_=cc_outs[hp][r])
            at_sb[2 * r + hp] = t

    # ---- phase 5: o_proj column slice ------------------------------------
    with tc.tile_pool(name="psO", bufs=2, space="PSUM") as psO:
        for sb_i in range(NSB):
            sl = slice(sb_i * 128, (sb_i + 1) * 128)
            o_ps = psO.tile([128, DQ], DT, name="o_ps", tag="o")
            for c in range(FCH):
                nc.tensor.matmul(o_ps, at_sb[c][:, sl], wo_sb[:, c, :],
                                 start=(c == 0), stop=(c == FCH - 1))
            o_sb = outp.tile([128, DQ], DT, name="o_sb", tag="o_sb")
            nc.scalar.copy(o_sb, o_ps)
            nc.sync.dma_start(out=out_s[sl, :], in_=o_sb)


def get_nc():
    if "nc" not in _NC_CACHE:
        _NC_CACHE["nc"] = _build_nc()
    return _NC_CACHE["nc"]


def make_in_maps(x, cos, sin, wq, wk, wv, wo):
    cosT = np.ascontiguousarray(np.asarray(cos, F32).T)
    sinT = np.ascontiguousarray(np.asarray(sin, F32).T)
    x = np.asarray(x, F32).astype(NPBF)
    wq, wk, wv, wo = (np.asarray(a, F32).astype(NPBF)
                      for a in (wq, wk, wv, wo))
    in_maps = []
    for core in range(NCORES):
        b, g = divmod(core, G)
        in_maps.append({
            "xT": np.ascontiguousarray(x[b].T),
            "cosT": cosT,
            "sinT": sinT,
            "wq": np.ascontiguousarray(wq[:, g * DQ:(g + 1) * DQ]),
            "wk": np.ascontiguousarray(wk[:, g * HD:(g + 1) * HD]),
            "wv": np.ascontiguousarray(wv[:, g * HD:(g + 1) * HD]),
            "wo": np.ascontiguousarray(wo[:, g * DQ:(g + 1) * DQ]),
        })
    return in_maps


def assemble(results):
    out = np.empty((B, S, D), F32)
    new_k = np.empty((B, S, KVH, HD), F32)
    new_v = np.empty((B, S, KVH, HD), F32)
    for core in range(NCORES):
        b, g = divmod(core, G)
        r = results[core]
        out[b, :, g * DQ:(g + 1) * DQ] = r["out_s"]
        new_k[b, :, g, :] = r["k_out"]
        new_v[b, :, g, :] = r["v_out"]
    return out, new_k, new_v


def _ensure_ntff_hook():
    """Register the axon NTFF profile hook if the container's antenv stub
    lacks it (needed only for trace=True timing runs)."""
    import sys
    import types
    try:
        from antenv.axon_hooks import get_axon_ntff_profile_hook  # noqa: F401
        return
    except ImportError:
        pass
    try:
        import antenv
        from trn_agent_boot.trn_boot import _ntff_profile_via_ctypes
        mod = types.ModuleType("antenv.axon_hooks")
        state = {"fn": None}
        mod.set_axon_ntff_profile_hook = lambda fn: state.update(fn=fn)
        mod.get_axon_ntff_profile_hook = lambda: state["fn"]
        sys.modules["antenv.axon_hooks"] = mod
        antenv.axon_hooks = mod
        hook = _ntff_profile_via_ctypes("/opt/axon/libaxon_pjrt.so")
        if hook is not None:
            mod.set_axon_ntff_profile_hook(hook)
    except Exception as e:  # profiling is best-effort; never break the run
        print(f"ntff hook setup failed: {e}")


def kernel(x, cos, sin, mask, wq, wk, wv, wo):
    # mask is not shipped to the device: the kernel applies causality
    # structurally, which matches the reference's -1e9 additive mask.
    nc = get_nc()
    in_maps = make_in_maps(x, cos, sin, wq, wk, wv, wo)
    trace = bool(int(os.environ.get("KERNEL_TRACE", "0")))
    if trace:
        _ensure_ntff_hook()
    res = run_bass_kernel_spmd(nc, in_maps, list(range(NCORES)), trace=trace)
    if trace:
        _NC_CACHE["last_exec_time_ns"] = res.exec_time_ns
    return assemble(res.results)


# revision 13
# speedup vs baseline: 2.7687x; 1.1841x over previous
"""Trainium2 Bass kernel for nn_Attention_47725676593424.

GQA attention layer: B=2, S=2048, D=1024, H=16 q-heads, KVH=4 kv-heads,
HD=64, RoPE, causal mask, returns (out, new_k, new_v).

Sharding (8 cores): core = b*4 + g, b = batch (data parallel), g = head
group (tensor parallel). Each core computes q-heads [4g, 4g+4) and kv
head g for batch b (whole KV group local, GQA repeat is implicit), then
AllGathers the per-head attention output within its 4-core batch group
and computes a 256-column slice of the o_proj output.

Layout: activations flow transposed (feature on partitions, sequence on
the free axis) so every matmul contracts on the partition dim with zero
on-device transposes of x. Scores are computed transposed S^T[k, q]; the
softmax denominator comes free from a ones-column appended to V. The
causal mask is applied structurally (upper-triangular key blocks are
skipped / zeroed), which matches the reference's additive -1e9 mask
exactly because exp underflows to 0. Softmax runs without max
subtraction: scores*scale is bounded (|s| < ~4) for any plausible
activation scale here, so exp cannot overflow.
"""

import os
import numpy as np
from contextlib import ExitStack

import concourse.bass as bass
import concourse.mybir as mybir
import concourse.tile as tile
from concourse import bacc
from concourse.bass_utils import run_bass_kernel_spmd
from concourse.masks import make_identity

# Problem constants (hardcoded per harness contract).
B, S, D = 2, 2048, 1024
H, KVH, HD = 16, 4, 64
NCORES = 8
G = 4                 # head groups (tensor-parallel degree per batch)
HPG = H // G          # 4 q heads per core
DQ = HPG * HD         # 256 = per-core q/attn feature dim
SCALE = 1.0 / 8.0     # 1/sqrt(HD)
PAN = 512             # q panel width (one PSUM bank of fp32)
NPAN = S // PAN       # 4
SB = 128              # s block
NSB = S // SB         # 16
FCH = D // 128        # 8 feature chunks of the contraction dim
DT = mybir.dt.float32
BF = mybir.dt.bfloat16
F32 = np.float32
try:
    import ml_dtypes
    NPBF = ml_dtypes.bfloat16
except ImportError:  # pragma: no cover
    NPBF = None

_NC_CACHE = {}


def _build_nc():
    nc = bacc.Bacc("TRN2", target_bir_lowering=False, debug=False,
                   num_devices=NCORES)

    xT_h = nc.dram_tensor("xT", [D, S], BF, kind="ExternalInput")
    cosT_h = nc.dram_tensor("cosT", [HD // 2, S], DT, kind="ExternalInput")
    sinT_h = nc.dram_tensor("sinT", [HD // 2, S], DT, kind="ExternalInput")
    wq_h = nc.dram_tensor("wq", [D, DQ], BF, kind="ExternalInput")
    wk_h = nc.dram_tensor("wk", [D, HD], BF, kind="ExternalInput")
    wv_h = nc.dram_tensor("wv", [D, HD], BF, kind="ExternalInput")
    wo_h = nc.dram_tensor("wo", [D, DQ], BF, kind="ExternalInput")
    out_h = nc.dram_tensor("out_s", [S, DQ], DT, kind="ExternalOutput")
    kout_h = nc.dram_tensor("k_out", [S, HD], DT, kind="ExternalOutput")
    vout_h = nc.dram_tensor("v_out", [S, HD], DT, kind="ExternalOutput")

    xT, cosT, sinT = xT_h.ap(), cosT_h.ap(), sinT_h.ap()
    wq, wk, wv, wo = wq_h.ap(), wk_h.ap(), wv_h.ap(), wo_h.ap()
    out_s, k_out, v_out = out_h.ap(), kout_h.ap(), vout_h.ap()

    with ExitStack() as ctx:
        tc = ctx.enter_context(tile.TileContext(nc))
        _emit(ctx, tc, nc, xT, cosT, sinT, wq, wk, wv, wo,
              out_s, k_out, v_out)

    nc.compile()
    return nc


def _emit(ctx, tc, nc, xT, cosT, sinT, wq, wk, wv, wo, out_s, k_out, v_out):
    EXP = mybir.ActivationFunctionType.Exp

    consts = ctx.enter_context(tc.tile_pool(name="consts", bufs=1))
    big = ctx.enter_context(tc.tile_pool(name="big", bufs=8))
    qkv = ctx.enter_context(tc.tile_pool(name="qkv", bufs=1))
    pt_pool = ctx.enter_context(tc.tile_pool(name="pt", bufs=3))
    tmp = ctx.enter_context(tc.tile_pool(name="tmp", bufs=6))
    outp = ctx.enter_context(tc.tile_pool(name="outp", bufs=3))
    dram = ctx.enter_context(tc.tile_pool(name="dram", bufs=1, space="DRAM"))

    # ---- constants / weights into SBUF ------------------------------------
    cosT_sb = consts.tile([HD // 2, S], DT)
    sinT_sb = consts.tile([HD // 2, S], DT)
    nc.sync.dma_start(out=cosT_sb, in_=cosT)
    nc.sync.dma_start(out=sinT_sb, in_=sinT)

    ident = consts.tile([128, 128], DT)
    make_identity(nc, ident)
    # 0/1 lower-triangle (keep q >= k) multiplier for diagonal score blocks
    tri01 = consts.tile([128, 128], BF)
    nc.gpsimd.memset(tri01, 1.0)
    nc.gpsimd.affine_select(
        out=tri01, in_=tri01, compare_op=mybir.AluOpType.is_ge,
        fill=0.0, base=0, pattern=[[1, 128]], channel_multiplier=-1)

    wq_sb = consts.tile([128, FCH, DQ], BF)
    wk_sb = consts.tile([128, FCH, HD], BF)
    wv_sb = consts.tile([128, FCH, HD], BF)
    wo_sb = consts.tile([128, FCH, DQ], BF)
    nc.sync.dma_start(out=wq_sb, in_=wq.rearrange("(c p) d -> p c d", p=128))
    nc.sync.dma_start(out=wk_sb, in_=wk.rearrange("(c p) d -> p c d", p=128))
    nc.sync.dma_start(out=wv_sb, in_=wv.rearrange("(c p) d -> p c d", p=128))
    nc.sync.dma_start(out=wo_sb, in_=wo.rearrange("(c p) d -> p c d", p=128))

    xt_sb = []
    for c in range(FCH):
        t = big.tile([128, S], BF, name=f"xt{c}", tag="big")
        nc.sync.dma_start(out=t, in_=xT[c * 128:(c + 1) * 128, :])
        xt_sb.append(t)

    # Persistent transposed activations.
    qT_sb = qkv.tile([128, 2, S], BF)       # 2 packs x (2 heads x 64)
    # k master in fp32 (feeds the k_out output); bf16 copy duplicated in
    # both partition halves so scores lhsT can match the base partition
    # (0 or 64) of each q head's rhs slice.
    kT_f32 = qkv.tile([HD, S], DT)
    kT_bf = qkv.tile([128, S], BF)
    vT_sb = qkv.tile([HD, S], DT)           # pre-transpose v (fp32 master)
    v_ext = qkv.tile([128, NSB, 65], DT)    # v natural + ones column
    v_ext_bf = qkv.tile([128, NSB, 65], BF)
    att0 = qkv.tile([128, S], BF)           # attn out^T, heads 0,1
    att1 = qkv.tile([128, S], BF)           # attn out^T, heads 2,3

    def rope_halves(dst, src, sp, base):
        """dst/src: [*, PAN] APs; rotate halves at partition base/base+32."""
        sl = slice(sp * PAN, (sp + 1) * PAN)
        c_sl = cosT_sb[:, sl]
        s_sl = sinT_sb[:, sl]
        a = slice(base, base + 32)
        b = slice(base + 32, base + 64)
        t1 = tmp.tile([32, PAN], DT, name="rt1", tag="ropet")
        t2 = tmp.tile([32, PAN], DT, name="rt2", tag="ropet")
        nc.vector.tensor_mul(t1, src[a], c_sl)
        nc.vector.tensor_mul(t2, src[b], s_sl)
        nc.gpsimd.tensor_sub(dst[a], t1, t2)
        t3 = tmp.tile([32, PAN], DT, name="rt3", tag="ropet")
        t4 = tmp.tile([32, PAN], DT, name="rt4", tag="ropet")
        nc.vector.tensor_mul(t3, src[b], c_sl)
        nc.vector.tensor_mul(t4, src[a], s_sl)
        nc.gpsimd.tensor_add(dst[b], t3, t4)

    # ---- phase 1: QKV projections + RoPE (own PSUM pool scope) ------------
    with tc.tile_pool(name="psA", bufs=2, space="PSUM") as psA:
        for pk in range(2):
            for sp in range(NPAN):
                q_ps = psA.tile([128, PAN], DT, name="q_ps", tag="ps")
                for c in range(FCH):
                    nc.tensor.matmul(
                        q_ps,
                        wq_sb[:, c, pk * 128:(pk + 1) * 128],
                        xt_sb[c][:, sp * PAN:(sp + 1) * PAN],
                        start=(c == 0), stop=(c == FCH - 1))
                dst = qT_sb[:, pk, sp * PAN:(sp + 1) * PAN]
                for hh in range(2):
                    rope_halves(dst, q_ps, sp, hh * 64)

        for sp in range(NPAN):
            k_ps = psA.tile([HD, PAN], DT, name="k_ps", tag="ps")
            for c in range(FCH):
                nc.tensor.matmul(
                    k_ps, wk_sb[:, c, :],
                    xt_sb[c][:, sp * PAN:(sp + 1) * PAN],
                    start=(c == 0), stop=(c == FCH - 1))
            rope_halves(kT_f32[:, sp * PAN:(sp + 1) * PAN], k_ps, sp, 0)
        nc.vector.tensor_copy(kT_bf[0:HD, :], kT_f32)
        nc.sync.dma_start(out=kT_bf[HD:128, :], in_=kT_bf[0:HD, :])

        for sp in range(NPAN):
            v_ps = psA.tile([HD, PAN], DT, name="v_ps", tag="ps")
            for c in range(FCH):
                nc.tensor.matmul(
                    v_ps, wv_sb[:, c, :],
                    xt_sb[c][:, sp * PAN:(sp + 1) * PAN],
                    start=(c == 0), stop=(c == FCH - 1))
            nc.scalar.copy(vT_sb[:, sp * PAN:(sp + 1) * PAN], v_ps)

        # ---- phase 2: k/v back to natural layout for outputs + AV --------
        nc.vector.memset(v_ext[:, :, 64:65], 1.0)
        for kb in range(NSB):
            sl = slice(kb * 128, (kb + 1) * 128)
            vt_ps = psA.tile([128, HD], DT, name="vt_ps", tag="tp")
            nc.tensor.transpose(vt_ps, vT_sb[:, sl], ident[0:HD, 0:HD])
            nc.vector.tensor_copy(v_ext[:, kb, 0:HD], vt_ps)
            nc.sync.dma_start(out=v_out[sl, :], in_=v_ext[:, kb, 0:HD])

            kt_ps = psA.tile([128, HD], DT, name="kt_ps", tag="tp")
            nc.tensor.transpose(kt_ps, kT_f32[:, sl], ident[0:HD, 0:HD])
            kn_sb = outp.tile([128, HD], DT, name="kn_sb", tag="kn")
            nc.vector.tensor_copy(kn_sb, kt_ps)
            nc.sync.dma_start(out=k_out[sl, :], in_=kn_sb)
        nc.vector.tensor_copy(v_ext_bf, v_ext)

    # ---- phase 3: causal attention, scores transposed --------------------
    # Head-pair outer so each half of attn^T finishes early and its
    # AllGather overlaps the other half's compute.
    cc_ins = [dram.tile([128, S], BF, name=f"cc_in{i}") for i in range(2)]
    cc_outs = [dram.tile([G, 128, S], BF, name=f"cc_out{i}") for i in range(2)]
    with tc.tile_pool(name="psS", bufs=2, space="PSUM") as psS, \
         tc.tile_pool(name="psAV", bufs=2, space="PSUM") as psAV:
        for hp in range(2):
            att = att0 if hp == 0 else att1
            for p in range(NPAN):
                nkb = 4 * (p + 1)
                q_sl = slice(p * PAN, (p + 1) * PAN)
                av_ps = psAV.tile([65, 2, PAN], DT, name="av_ps", tag="av")
                for kb in range(nkb):
                    k_sl = slice(kb * 128, (kb + 1) * 128)
                    off = (kb - 4 * p) * 128  # >=0 only on diagonal blocks
                    lo = max(off, 0)  # first valid q column in this panel
                    s_ps = psS.tile([128, 2, PAN], DT, name="s_ps", tag="s")
                    for hi in range(2):
                        base = hi * 64
                        nc.tensor.matmul(
                            s_ps[:, hi, lo:],
                            kT_bf[base:base + 64, k_sl],
                            qT_sb[base:base + 64, hp,
                                  p * PAN + lo:(p + 1) * PAN],
                            start=True, stop=True)
                    pt = pt_pool.tile([128, 2, PAN], BF, name="pt", tag="pt")
                    nc.scalar.activation(pt[:, :, lo:], s_ps[:, :, lo:],
                                         EXP, scale=SCALE)
                    if off >= 0:
                        for hi in range(2):
                            # keep q >= k within the diagonal 128-block
                            nc.vector.tensor_mul(
                                pt[:, hi, off:off + 128],
                                pt[:, hi, off:off + 128], tri01)
                    for hi in range(2):
                        nc.tensor.matmul(
                            av_ps[:, hi, lo:], v_ext_bf[:, kb, :],
                            pt[:, hi, lo:],
                            start=(kb == 0), stop=(kb == nkb - 1))
                z_sb = tmp.tile([1, 2, PAN], DT, name="z_sb", tag="z")
                nc.scalar.copy(z_sb, av_ps[64:65, :, :])
                r_sb = tmp.tile([1, 2, PAN], DT, name="r_sb", tag="r")
                nc.vector.reciprocal_approx_fast(out=r_sb, in_=z_sb)
                # broadcast 1/Z across partitions via a tiny DRAM bounce
                # (engines can't partition-broadcast; gpsimd must stay free
                # for the collectives)
                r_dr = dram.tile([1, 2, PAN], DT, name="r_dr", tag="r_dr",
                                 bufs=2)
                nc.sync.dma_start(out=r_dr, in_=r_sb)
                for hi in range(2):
                    rb = tmp.tile([HD, PAN], DT, name="rb", tag="rb")
                    nc.sync.dma_start(
                        out=rb, in_=r_dr[0:1, hi, :].to_broadcast([HD, PAN]))
                    nc.vector.tensor_mul(
                        att[hi * 64:hi * 64 + 64, q_sl],
                        av_ps[0:HD, hi, :], rb)
            # ship this head-pair's attn^T while the next pair computes
            nc.sync.dma_start(out=cc_ins[hp], in_=att)
            nc.gpsimd.collective_compute(
                "AllGather", mybir.AluOpType.bypass,
                replica_groups=[[0, 1, 2, 3], [4, 5, 6, 7]],
                ins=[cc_ins[hp].opt()], outs=[cc_outs[hp].opt()])

    # cc_out[hp][r] holds heads {4r+2hp, 4r+2hp+1} = wo chunk 2r+hp.
    at_sb = [None] * FCH
    for hp in range(2):
        for r in range(G):
            t = big.tile([128, S], BF, name=f"at{hp}_{r}", tag="big")
            nc.sync.dma_start(out=t, in_=cc_outs[hp][r])
            at_sb[2 * r + hp] = t

    # ---- phase 5: o_proj column slice ------------------------------------
    with tc.tile_pool(name="psO", bufs=2, space="PSUM") as psO:
        for sb_i in range(NSB):
            sl = slice(sb_i * 128, (sb_i + 1) * 128)
            o_ps = psO.tile([128, DQ], DT, name="o_ps", tag="o")
            for c in range(FCH):
                nc.tensor.matmul(o_ps, at_sb[c][:, sl], wo_sb[:, c, :],
                                 start=(c == 0), stop=(c == FCH - 1))
            o_sb = outp.tile([128, DQ], DT, name="o_sb", tag="o_sb")
            nc.scalar.copy(o_sb, o_ps)
            nc.sync.dma_start(out=out_s[sl, :], in_=o_sb)


def get_nc():
    if "nc" not in _NC_CACHE:
        _NC_CACHE["nc"] = _build_nc()
    return _NC_CACHE["nc"]


def make_in_maps(x, cos, sin, wq, wk, wv, wo):
    cosT = np.ascontiguousarray(np.asarray(cos, F32).T)
    sinT = np.ascontiguousarray(np.asarray(sin, F32).T)
    x = np.asarray(x, F32).astype(NPBF)
    wq, wk, wv, wo = (np.asarray(a, F32).astype(NPBF)
                      for a in (wq, wk, wv, wo))
    in_maps = []
    for core in range(NCORES):
        b, g = divmod(core, G)
        in_maps.append({
            "xT": np.ascontiguousarray(x[b].T),
            "cosT": cosT,
            "sinT": sinT,
            "wq": np.ascontiguousarray(wq[:, g * DQ:(g + 1) * DQ]),
            "wk": np.ascontiguousarray(wk[:, g * HD:(g + 1) * HD]),
            "wv": np.ascontiguousarray(wv[:, g * HD:(g + 1) * HD]),
            "wo": np.ascontiguousarray(wo[:, g * DQ:(g + 1) * DQ]),
        })
    return in_maps


def assemble(results):
    out = np.empty((B, S, D), F32)
    new_k = np.empty((B, S, KVH, HD), F32)
    new_v = np.empty((B, S, KVH, HD), F32)
    for core in range(NCORES):
        b, g = divmod(core, G)
        r = results[core]
        out[b, :, g * DQ:(g + 1) * DQ] = r["out_s"]
        new_k[b, :, g, :] = r["k_out"]
        new_v[b, :, g, :] = r["v_out"]
    return out, new_k, new_v


def _ensure_ntff_hook():
    """Register the axon NTFF profile hook if the container's antenv stub
    lacks it (needed only for trace=True timing runs)."""
    import sys
    import types
    try:
        from antenv.axon_hooks import get_axon_ntff_profile_hook  # noqa: F401
        return
    except ImportError:
        pass
    try:
        import antenv
        from trn_agent_boot.trn_boot import _ntff_profile_via_ctypes
        mod = types.ModuleType("antenv.axon_hooks")
        state = {"fn": None}
        mod.set_axon_ntff_profile_hook = lambda fn: state.update(fn=fn)
        mod.get_axon_ntff_profile_hook = lambda: state["fn"]
        sys.modules["antenv.axon_hooks"] = mod
        antenv.axon_hooks = mod
        hook = _ntff_profile_via_ctypes("/opt/axon/libaxon_pjrt.so")
        if hook is not None:
            mod.set_axon_ntff_profile_hook(hook)
    except Exception as e:  # profiling is best-effort; never break the run
        print(f"ntff hook setup failed: {e}")


def kernel(x, cos, sin, mask, wq, wk, wv, wo):
    # mask is not shipped to the device: the kernel applies causality
    # structurally, which matches the reference's -1e9 additive mask.
    nc = get_nc()
    in_maps = make_in_maps(x, cos, sin, wq, wk, wv, wo)
    trace = bool(int(os.environ.get("KERNEL_TRACE", "0")))
    if trace:
        _ensure_ntff_hook()
    res = run_bass_kernel_spmd(nc, in_maps, list(range(NCORES)), trace=trace)
    if trace:
        _NC_CACHE["last_exec_time_ns"] = res.exec_time_ns
    return assemble(res.results)
